# revision 23
# baseline (speedup 1.0000x reference)
"""Causal self-attention with ALiBi for TRN2, 8 NeuronCores.

Sharding: core c -> batch b = c % 4, head-shard hs = c // 4.
Head-shard hs owns global heads {2j + hs : j in 0..7} (interleaved so both
shards see the same mix of ALiBi slopes -> balanced banded-attention work).

Key HW facts this kernel is built around (measured on TRN2):
  * bf16 matmuls issue at N/2.4GHz cols with LDWEIGHTS overlapped; the PE
    supports row-group tiling (tile_position), so the two heads of a pair
    run their K=64-contraction score matmuls CONCURRENTLY in disjoint
    64-row halves of the array (~1.9x on scores).
  * the HAM clock gate runs the PE at 1.2 GHz until ~3.4us of sustained
    activity and re-gates after any ~3.4us idle window; the kernel keeps
    one continuous PE instruction stream (warm-up burst -> DMA-paced
    d-outer first projection -> paced attention/filler interleave).
  * ACT exp costs (w + 352)/1.2 ns per op; total exp (~124us) is nearly
    co-critical with the PE stream (~160us), so attention score/exp groups
    are spread across the WHOLE kernel, paced against dense matmul filler
    units (remaining projections, V stripes, out-proj stripes).
  * reciprocal_approx_fast CANNOT read PSUM directly (garbage, no error):
    the denominator row is staged through SBUF first.

Per-core computation (B=1 batch, 8 heads):
  V is produced in [s, col] layout with a ones column appended per head
  (the PV matmul then yields both the unnormalized output AND the softmax
  denominator).  Per pair p (slots 2p, 2p+1): Q^T/K^T in [col, s] layout,
  head pairs packed 64+64 into 128-partition tiles (Q pre-scaled by
  1/sqrt(HD) via host-scaled Wq).  Scores S^T[k,q] = K_h^T.T @ Q_h^T with
  64-row tile_position per half, exp on ACT with per-partition bias
  slope*(k - qmid) (band truncation at CUT skips vanishing k-tiles),
  PV accumulation into PSUM [65, 512] over the k band, then normalize
  (reciprocal of row 64, broadcast via gpsimd) into OT (bf16).  Pairs run
  heavy-to-light (3,2,1,0); out-projection stripes interleave into the
  last attention window per q-chunk.

Host side: shard/transpose/bf16-convert/pack inputs, run SPMD on 8 cores,
sum the two head-shards' partial outputs per batch, add bo.
"""

import math

import numpy as np

B, S, D, H = 4, 2048, 1024, 16
HD = D // H
NSLOT = 8          # local heads per core
NQC = 4            # q chunks of 512
NKT = 16           # k tiles of 128
SC = 512
KT = 128
NCORES = 8

# ALiBi slopes for global heads
SLOPES = [2.0 ** (-0.5 * (h + 1)) for h in range(H)]

# band cutoff: terms with slope*(q-k) > CUT are < e^-CUT relative to the
# diagonal term and invisible next to the bf16 matmul noise (~4e-3)
CUT = 8.0


def _bt(h):
    """Band width in 128-k-tiles for global head h (delta_max + 1)."""
    d_max = int(math.ceil(CUT / SLOPES[h]))
    return min(NKT, (127 + d_max) // 128 + 1)


def _w(h):
    """Max exp-op width (q columns) for global head h: slope*(W/2) <= 64
    (bounds the exp dynamic range across a recentered column block)."""
    s = SLOPES[h]
    if s * 256.0 <= 64.0:
        return 512
    if s * 128.0 <= 64.0:
        return 256
    return 128


# per-slot params = union over the two head shards (program is SPMD-shared)
SLOT_BT = [max(_bt(2 * j), _bt(2 * j + 1)) for j in range(NSLOT)]
SLOT_W = [min(_w(2 * j), _w(2 * j + 1)) for j in range(NSLOT)]


def plan_attention():
    """Enumerate all attention tile ops. Returns (ops, bias_cols) where ops is
    a list of dicts and bias_cols maps (slot, mkey) -> expb column index."""
    bias_cols = {}
    ops = []
    for p in range(4):
        for qc in range(NQC):
            for kt in range(4 * qc + 4):
                for half in (0, 1):
                    j = 2 * p + half
                    bt, w = SLOT_BT[j], SLOT_W[j]
                    lo = max(0, 4 * qc - bt + 1)
                    if kt < lo:
                        continue
                    qs_start = max(4 * qc, kt)
                    qs_end = min(4 * qc + 3, kt + bt - 1)
                    if qs_start > qs_end:
                        continue
                    c0 = 128 * (qs_start - 4 * qc)
                    c1 = 128 * (qs_end - 4 * qc) + 128
                    # exp ops aligned to an absolute w-grid within the qc
                    # chunk: qmid (the recentering constant) must depend only
                    # on the column block, never on kt, so that every term
                    # entering a given column's softmax sum carries the same
                    # exp(-slope*qmid) factor.
                    exps = []
                    for g in range((c0 // w) * w, c1, w):
                        a, e = max(c0, g), min(c1, g + w)
                        if a >= e:
                            continue
                        mkey = (512 * qc + g + w // 2) - 128 * kt
                        col = bias_cols.setdefault((j, mkey), len(bias_cols))
                        exps.append((a, e - a, col))
                    ops.append(dict(qc=qc, p=p, half=half, j=j, kt=kt,
                                    c0=c0, c1=c1, exps=exps,
                                    tril=(kt >= 4 * qc),
                                    first=(kt == lo), last=(kt == 4 * qc + 3)))
    return ops, bias_cols


ATT_OPS, BIAS_COLS = plan_attention()
NBIAS = len(BIAS_COLS)

# xT tile consumption order for the DMA-paced startup pass (approximate
# arrival order given the 3-queue round-robin below)
D_ORDER = [0, 1, 3, 4, 2, 6, 7, 5]

_nc_cache = None


def build_program():
    global _nc_cache
    if _nc_cache is not None:
        return _nc_cache

    import concourse.bacc as bacc
    import concourse.tile as tile
    from concourse import mybir

    F32 = mybir.dt.float32
    BF16 = mybir.dt.bfloat16
    EXP = mybir.ActivationFunctionType.Exp
    COPY = mybir.ActivationFunctionType.Copy

    nc = bacc.Bacc("TRN2", target_bir_lowering=False, debug=False,
                   num_devices=NCORES)

    xT_d = nc.dram_tensor("xT", [D, S], BF16, kind="ExternalInput")
    # packed weights: one DMA each.  wqk[p]: [128, d(8) x (Q128 | K128)];
    # wvp: [128, d(8) x 512 vcols]; wop: [128, f(4) x 1024 dcols]
    wqk_d = [nc.dram_tensor(f"wqk{p}", [128, 2048], BF16,
                            kind="ExternalInput") for p in range(4)]
    wvp_d = nc.dram_tensor("wvp", [128, 4096], BF16, kind="ExternalInput")
    wop_d = nc.dram_tensor("wop", [128, 4096], BF16, kind="ExternalInput")
    qkb_d = nc.dram_tensor("qkb", [128, 8], F32, kind="ExternalInput")
    bvr_d = nc.dram_tensor("bvr", [128, 512], F32, kind="ExternalInput")
    expb_d = nc.dram_tensor("expb", [128, max(NBIAS, 1)], F32,
                            kind="ExternalInput")
    tril_d = nc.dram_tensor("tril", [128, 128], BF16, kind="ExternalInput")
    vones_d = nc.dram_tensor("vones", [128, 8], BF16, kind="ExternalInput")
    out_d = nc.dram_tensor("out_p", [S, D], BF16, kind="ExternalOutput")

    ops_by_pqc = {}
    for o in ATT_OPS:
        ops_by_pqc.setdefault((o["p"], o["qc"]), []).append(o)

    with tile.TileContext(nc) as tc:
        with nc.allow_low_precision(reason="bf16 attention kernel"), \
             tc.tile_pool(name="persist", bufs=1) as pp, \
             tc.tile_pool(name="expsp", bufs=8) as expsp, \
             tc.tile_pool(name="rcp", bufs=2) as rcp, \
             tc.tile_pool(name="rbp", bufs=2) as rbp, \
             tc.tile_pool(name="outp", bufs=4) as outp, \
             tc.tile_pool(name="qps", bufs=2, space="PSUM") as qps, \
             tc.tile_pool(name="sps", bufs=3, space="PSUM") as sps, \
             tc.tile_pool(name="ops_", bufs=3, space="PSUM") as ops_:

            # ---- persistent tiles ----
            qkT_Q = [pp.tile([128, S], BF16, name=f"qkTQ{p}") for p in range(4)]
            # packed K stationaries: head h of pair p lives in partitions
            # 64h..64h+63 (the natural projection layout); score matmuls are
            # 64-row tile_position'd so both heads run concurrently.
            kqT = [pp.tile([128, S], BF16, name=f"kqT{p}") for p in range(4)]
            Vbuf = [pp.tile([128, NSLOT * 65], BF16, name=f"vb{t}")
                    for t in range(NKT)]
            OT = [pp.tile([128, S], BF16, name=f"OT{p}") for p in range(4)]
            xT = [pp.tile([128, S], BF16, name=f"xT{d}") for d in range(8)]
            wqk_t = [pp.tile([128, 2048], BF16, name=f"wqk_t{p}")
                     for p in range(4)]
            wv = pp.tile([128, 4096], BF16, name="wv")
            wo_t = pp.tile([128, 4096], BF16, name="wo_t")
            qkb_t = pp.tile([128, 8], F32, name="qkb_t")
            bvr_t = pp.tile([128, 512], F32, name="bvr_t")
            expb_t = pp.tile([128, max(NBIAS, 1)], F32, name="expb_t")
            tril_t = pp.tile([128, 128], BF16, name="tril_t")
            vones_t = pp.tile([128, 8], BF16, name="vones_t")
            warm_t = pp.tile([128, 128], BF16, name="warm_t")
            dume = pp.tile([128, 1], BF16, name="dume")

            # ---- input DMA: xT tiles lead all three queues (they gate the
            # startup projection pass); small tensors follow; wop last.
            nc.sync.dma_start(out=wqk_t[3], in_=wqk_d[3][:, :])
            for d in [0, 3, 6]:
                nc.gpsimd.dma_start(out=xT[d], in_=xT_d[128 * d:128 * (d + 1), :])
            for d in [1, 4, 7]:
                nc.scalar.dma_start(out=xT[d], in_=xT_d[128 * d:128 * (d + 1), :])
            for d in [2, 5]:
                nc.sync.dma_start(out=xT[d], in_=xT_d[128 * d:128 * (d + 1), :])
            nc.gpsimd.dma_start(out=qkb_t, in_=qkb_d[:, :])
            nc.gpsimd.dma_start(out=expb_t, in_=expb_d[:, :])
            nc.gpsimd.dma_start(out=vones_t, in_=vones_d[:, :])
            nc.gpsimd.dma_start(out=tril_t, in_=tril_d[:, :])
            nc.sync.dma_start(out=wv, in_=wvp_d[:, :])
            nc.gpsimd.dma_start(out=bvr_t, in_=bvr_d[:, :])
            nc.scalar.dma_start(out=wqk_t[2], in_=wqk_d[2][:, :])
            nc.sync.dma_start(out=wqk_t[1], in_=wqk_d[1][:, :])
            nc.gpsimd.dma_start(out=wqk_t[0], in_=wqk_d[0][:, :])
            nc.scalar.dma_start(out=wo_t, in_=wop_d[:, :])

            # ---- PE warm-up on a memset tile (no DMA dependency): keeps the
            # HAM clock gate fed from ~0.4us until the first xT tiles land.
            nc.vector.memset(warm_t, 0.0)
            for _ in range(24):
                wps = qps.tile([128, 128], F32, name="warm", tag="q")
                nc.tensor.matmul(wps, warm_t, warm_t, start=True, stop=True)
            # preload the ACT exp table set during the DMA window
            nc.scalar.activation(dume, warm_t[:, 0:1], EXP,
                                 bias=0.0, scale=1.0)

            # ones columns of Vbuf (col 64 of each 65-wide head group)
            for t in range(NKT):
                ones_view = Vbuf[t].rearrange("p (h c) -> p h c", c=65)[:, :, 64:65]
                nc.vector.tensor_copy(ones_view, vones_t.unsqueeze(2))

            # ================= emission units =================

            def proj_fin(p, m, si_abs, psq):
                """Move one [128,512] projection PSUM tile to SBUF (+bias)."""
                if m == 0:
                    nc.vector.tensor_scalar_add(
                        qkT_Q[p][:, SC * si_abs:SC * (si_abs + 1)], psq,
                        qkb_t[:, p:p + 1])
                else:
                    nc.vector.tensor_scalar_add(
                        kqT[p][:, SC * si_abs:SC * (si_abs + 1)], psq,
                        qkb_t[:, 4 + p:5 + p])

            def proj_pass_start(p, si_abs):
                """DMA-paced d-outer Q+K projection for s-chunk si_abs: both
                tiles accumulate as each xT[d] lands (startup only)."""
                psqK = qps.tile([128, SC], F32, name="psqK", tag="q")
                psqQ = qps.tile([128, SC], F32, name="psqQ", tag="q")
                for i, d in enumerate(D_ORDER):
                    mv = xT[d][:, SC * si_abs:SC * (si_abs + 1)]
                    nc.tensor.matmul(
                        psqK, wqk_t[p][:, 256 * d + 128:256 * d + 256], mv,
                        start=(i == 0), stop=(i == 7), skip_group_check=True)
                    nc.tensor.matmul(
                        psqQ, wqk_t[p][:, 256 * d:256 * d + 128], mv,
                        start=(i == 0), stop=(i == 7), skip_group_check=True)
                    # keep the HAM activity monitor fed through the DMA-paced
                    # gaps between xT arrivals (sustained-busy -> 2.4 GHz by
                    # the time attention starts)
                    if i >= 1:
                        # ops_ pool is idle during startup; its banks host the
                        # gap-filling warm tiles (psq tiles hold qps' slots).
                        # These only run when the next xT tile hasn't landed
                        # (the scheduler orders by readiness), so oversizing
                        # is cheap.
                        for _ in range(4):
                            wps = ops_.tile([128, 128], F32, name="warm2",
                                            tag="psumO")
                            nc.tensor.matmul(wps, warm_t, warm_t,
                                             start=True, stop=True)
                proj_fin(p, 1, si_abs, psqK)
                proj_fin(p, 0, si_abs, psqQ)

            def proj_unit(p, m, si_abs):
                """Classic d-inner projection unit: one [128,512] output."""
                psq = qps.tile([128, SC], F32, name="psq", tag="q")
                for d in range(8):
                    nc.tensor.matmul(
                        psq, wqk_t[p][:, 256 * d + 128 * m:256 * d + 128 * (m + 1)],
                        xT[d][:, SC * si_abs:SC * (si_abs + 1)],
                        start=(d == 0), stop=(d == 7))
                proj_fin(p, m, si_abs, psq)

            def v_unit(st):
                """V projection stripe st into Vbuf[st] (bf16, [s,col])."""
                psv = qps.tile([128, SC], F32, name="psv", tag="q")
                for d in range(8):
                    nc.tensor.matmul(
                        psv, xT[d][:, 128 * st:128 * (st + 1)],
                        wv[:, 512 * d:512 * (d + 1)],
                        start=(d == 0), stop=(d == 7))
                vdst = Vbuf[st].rearrange("p (h c) -> p h c", c=65)[:, :, 0:64]
                nc.vector.tensor_tensor(
                    vdst, psv.rearrange("p (g c) -> p g c", c=64),
                    bvr_t.rearrange("p (g c) -> p g c", c=64),
                    op=mybir.AluOpType.add)

            def out_unit(st, act_free):
                """Out-projection stripe st: psum accumulate over 4 pairs,
                then copy+DMA per 512-col half."""
                pse = [qps.tile([128, SC], F32, name="pse", tag="q")
                       for _ in range(2)]
                for d in range(4):
                    for e in range(2):
                        nc.tensor.matmul(
                            pse[e], OT[d][:, 128 * st:128 * (st + 1)],
                            wo_t[:, 1024 * d + SC * e:1024 * d + SC * (e + 1)],
                            start=(d == 0), stop=(d == 3))
                for e in range(2):
                    ob = outp.tile([128, SC], BF16, name="ob")
                    if act_free and e == 1:
                        nc.scalar.activation(ob, pse[e], COPY)
                    else:
                        nc.vector.tensor_copy(ob, pse[e])
                    eng = nc.gpsimd if (st + e) % 2 == 0 else nc.sync
                    eng.dma_start(
                        out=out_d[128 * st:128 * (st + 1),
                                  SC * e:SC * (e + 1)],
                        in_=ob)

            tril_ctr = [0]

            def emit_scores(grp):
                """Row-tiled score matmuls + exp for one kt group (both
                halves concurrent in disjoint 64-row PE tiles)."""
                out = []
                p = grp[0]["p"]
                for o in grp:
                    half, kt = o["half"], o["kt"]
                    c0, c1 = o["c0"], o["c1"]
                    qc = o["qc"]
                    hb = 64 * half
                    psS = sps.tile([128, SC], F32, name="psS")
                    nc.tensor.matmul(
                        psS[:, c0:c1],
                        kqT[p][hb:hb + 64, 128 * kt:128 * (kt + 1)],
                        qkT_Q[p][hb:hb + 64, SC * qc + c0:SC * qc + c1],
                        start=True, stop=True)
                    eS = expsp.tile([128, SC], BF16, name="eS")
                    for (a, ww, col) in o["exps"]:
                        nc.scalar.activation(
                            eS[:, a:a + ww], psS[:, a:a + ww], EXP,
                            bias=expb_t[:, col:col + 1], scale=1.0)
                    if o["tril"]:
                        # NOTE: must stay on vector — gpsimd tensor_tensor
                        # lives in a different ucode library than
                        # partition_broadcast, and alternating them costs
                        # ~6.5us per LOAD_LIB swap.
                        nc.vector.tensor_mul(
                            eS[:, c0:c0 + 128], eS[:, c0:c0 + 128], tril_t)
                    out.append((o, eS))
                return out

            def emit_pv(ready, psumO):
                for (o, eS) in ready:
                    c0, c1 = o["c0"], o["c1"]
                    nc.tensor.matmul(
                        psumO[o["half"]][0:65, c0:c1],
                        Vbuf[o["kt"]][:, 65 * o["j"]:65 * o["j"] + 65],
                        eS[:, c0:c1],
                        start=o["first"], stop=o["last"])

            def emit_norm(p, qc, psumO):
                # both halves' chains pipelined: copies, then recips, then
                # broadcasts, then muls (keeps chain latency ~2 stages, not 6)
                ss = {}
                for half in (0, 1):
                    ss[half] = rcp.tile([1, SC], F32, name="ssum")
                    nc.vector.tensor_copy(ss[half], psumO[half][64:65, :])
                rc = {}
                for half in (0, 1):
                    rc[half] = rcp.tile([1, SC], F32, name="rc")
                    nc.vector.reciprocal_approx_fast(rc[half], ss[half])
                rb = {}
                for half in (0, 1):
                    rb[half] = rbp.tile([64, SC], F32, name="rb")
                    nc.gpsimd.partition_broadcast(rb[half], rc[half])
                for half in (0, 1):
                    nc.vector.tensor_mul(
                        OT[p][64 * half:64 * half + 64,
                              SC * qc:SC * (qc + 1)],
                        psumO[half][0:64, :],
                        rb[half])

            # ================= cost model for pacing =================

            def grp_costs(grp):
                pe = 0.0
                act = 0.0
                bycols = {0: 0, 1: 0}
                for o in grp:
                    bycols[o["half"]] = o["c1"] - o["c0"]
                    for (a, ww, col) in o["exps"]:
                        act += (ww + 352) / 1.2
                pe = max(bycols.values()) / 2.4 + 110.0
                return pe, act

            def pv_costs(grp):
                return sum(o["c1"] - o["c0"] for o in grp) / 2.4 + 60.0 * len(grp)

            UNIT_PE = 8 * (512 / 2.4 + 45.0)   # proj/v/out unit estimate

            # ================= filler list =================
            # (emit_fn, pe_ns); consumed in order by the pacer
            fillers = []
            for si_abs in (1, 2, 3):
                fillers.append((lambda s=si_abs: proj_unit(3, 1, s), UNIT_PE))
                fillers.append((lambda s=si_abs: proj_unit(3, 0, s), UNIT_PE))
            for st in range(NKT):
                fillers.append((lambda s=st: v_unit(s), UNIT_PE))
            for p_ in (2, 1, 0):
                for si_abs in range(4):
                    fillers.append((lambda pp_=p_, s=si_abs: proj_unit(pp_, 1, s),
                                    UNIT_PE))
                    fillers.append((lambda pp_=p_, s=si_abs: proj_unit(pp_, 0, s),
                                    UNIT_PE))

            # filler force-markers
            FPROJ3 = 6            # proj pair-3 remainder
            FV = FPROJ3 + NKT     # V stripes end index (6..21)
            FP2 = FV + 8
            FP1 = FP2 + 8
            FP0 = FP1 + 8

            state = dict(fi=0, pe=0.0, act=1.0)

            # overall PE/ACT balance ratio for the pacer
            tot_act = sum(grp_costs([o])[1] for o in ATT_OPS)
            tot_pe = sum(f[1] for f in fillers) + 16 * UNIT_PE
            for (pq, opl) in ops_by_pqc.items():
                gs = {}
                for o in opl:
                    gs.setdefault(o["kt"], []).append(o)
                for g in gs.values():
                    tot_pe += grp_costs(g)[0] + pv_costs(g)
            RATIO = tot_pe / tot_act

            def force_fill(upto):
                while state["fi"] < upto:
                    fn, pe_ns = fillers[state["fi"]]
                    state["fi"] += 1
                    fn()
                    state["pe"] += pe_ns

            def pace_fill():
                while (state["fi"] < len(fillers)
                       and state["pe"] < state["act"] * RATIO):
                    fn, pe_ns = fillers[state["fi"]]
                    state["fi"] += 1
                    fn()
                    state["pe"] += pe_ns

            def attention_qc(p, qc, req_scores, req_pv_base):
                """Emit one (pair, qc) attention chunk through the pacer."""
                opl = ops_by_pqc.get((p, qc), [])
                groups = []
                for o in opl:
                    if groups and groups[-1][0]["kt"] == o["kt"]:
                        groups[-1].append(o)
                    else:
                        groups.append([o])
                psumO = {h: ops_.tile([65, SC], F32, name="psumO")
                         for h in (0, 1)}
                PIPE = 3
                pend = []
                force_fill(req_scores)
                for grp in groups:
                    pace_fill()
                    pend.append(emit_scores(grp))
                    pe, act = grp_costs(grp)
                    state["pe"] += pe
                    state["act"] += act
                    if len(pend) > PIPE:
                        ready = pend.pop(0)
                        if req_pv_base is not None:
                            force_fill(min(req_pv_base + ready[0][0]["kt"] + 1,
                                           FV))
                        pace_fill()
                        emit_pv(ready, psumO)
                        state["pe"] += pv_costs([o for (o, _) in ready])
                for ready in pend:
                    if req_pv_base is not None:
                        force_fill(min(req_pv_base + ready[0][0]["kt"] + 1, FV))
                    pace_fill()
                    emit_pv(ready, psumO)
                    state["pe"] += pv_costs([o for (o, _) in ready])
                emit_norm(p, qc, psumO)

            # ================= schedule =================

            # DMA-paced startup: Q+K projection of pair 3 for s-chunk 0
            proj_pass_start(3, 0)

            # pair 3 window (fillers: proj3 remainder, V stripes, proj2...)
            REQ3 = {0: 0, 1: 2, 2: 4, 3: 6}
            for qc in range(NQC):
                attention_qc(3, qc, REQ3[qc], FPROJ3)
            # pair 2 window
            for qc in range(NQC):
                attention_qc(2, qc, FP2, None)
            # pairs 1+0 interleaved by qc; qc0 (the lightest chunk) runs LAST
            # so the tail chain (exp->PV->norm->out) is as short as possible,
            # with the previous qc's out stripes as PE fill between the two
            # pairs of the next qc.
            QC_ORDER = [1, 2, 3, 0]
            prev_qc = None
            for qc in QC_ORDER:
                attention_qc(1, qc, FP1, None)
                if prev_qc is not None:
                    for st in range(4 * prev_qc, 4 * prev_qc + 4):
                        out_unit(st, act_free=False)
                        state["pe"] += UNIT_PE
                attention_qc(0, qc, FP0, None)
                prev_qc = qc
            for st in range(0, 4):
                out_unit(st, act_free=True)
                state["pe"] += UNIT_PE
            force_fill(len(fillers))

    nc.compile()
    _nc_cache = nc
    return nc


def make_inputs(x, mask, Wqkv, bqkv, Wo, bo):
    """Build the 8 per-core input maps."""
    import ml_dtypes

    bf16 = ml_dtypes.bfloat16
    x = np.asarray(x, dtype=np.float32)
    Wqkv = np.asarray(Wqkv, dtype=np.float32)
    bqkv = np.asarray(bqkv, dtype=np.float32)
    Wo = np.asarray(Wo, dtype=np.float32)

    # diagonal-block mask in [k_partition, q_column] layout: keep k <= q,
    # i.e. partition p <= column c -> UPPER-triangular
    tril = np.triu(np.ones((128, 128), dtype=bf16))
    vones = np.ones((128, 8), dtype=bf16)
    p_idx = np.arange(128, dtype=np.float32)[:, None]

    in_maps = []
    for c in range(NCORES):
        b, hs = c % 4, c // 4
        heads = [2 * j + hs for j in range(NSLOT)]

        # per-pair packed QK weights: [128, d(8) x (Q128 | K128)]
        # pair p covers slots 2p (partitions 0-63) and 2p+1 (64-127).
        wqk = {}
        for p in range(4):
            h0, h1 = heads[2 * p], heads[2 * p + 1]
            qcols = np.concatenate(
                [np.arange(h0 * HD, h0 * HD + HD),
                 np.arange(h1 * HD, h1 * HD + HD)])
            kcols = D + qcols
            wq = Wqkv[:, qcols] * 0.125            # [D, 128]
            wk = Wqkv[:, kcols]
            pack = np.empty((128, 2048), dtype=np.float32)
            for d in range(8):
                pack[:, 256 * d:256 * d + 128] = wq[128 * d:128 * (d + 1)]
                pack[:, 256 * d + 128:256 * d + 256] = wk[128 * d:128 * (d + 1)]
            wqk[p] = np.ascontiguousarray(pack).astype(bf16)

        # packed V weights [128, d(8) x 512] and Wo [128, f(4) x 1024]
        vcols = np.concatenate(
            [np.arange(2 * D + h * HD, 2 * D + h * HD + HD) for h in heads])
        wvm = Wqkv[:, vcols]                       # [D, 512]
        wvp = np.empty((128, 4096), dtype=np.float32)
        for d in range(8):
            wvp[:, 512 * d:512 * (d + 1)] = wvm[128 * d:128 * (d + 1)]
        rows = np.concatenate(
            [np.arange(h * HD, h * HD + HD) for h in heads])
        wom = Wo[rows, :]                          # [512, 1024]
        wop = np.empty((128, 4096), dtype=np.float32)
        for d in range(4):
            wop[:, 1024 * d:1024 * (d + 1)] = wom[128 * d:128 * (d + 1)]

        # biases: qkb col m -> per-partition bias for (Q pairs 0-3, K pairs
        # 0-3); pair p partitions = slot 2p dims then slot 2p+1 dims
        bq = np.empty((128, 8), dtype=np.float32)
        for p in range(4):
            h0, h1 = heads[2 * p], heads[2 * p + 1]
            qb = np.concatenate([bqkv[h0 * HD:h0 * HD + HD],
                                 bqkv[h1 * HD:h1 * HD + HD]]) * 0.125
            kb = np.concatenate([bqkv[D + h0 * HD:D + h0 * HD + HD],
                                 bqkv[D + h1 * HD:D + h1 * HD + HD]])
            bq[:, p] = qb
            bq[:, 4 + p] = kb
        bvr = np.broadcast_to(bqkv[2 * D:][vcols - 2 * D], (128, 512)).copy()

        expb = np.zeros((128, max(NBIAS, 1)), dtype=np.float32)
        for (j, mkey), col in BIAS_COLS.items():
            expb[:, col:col + 1] = SLOPES[2 * j + hs] * (p_idx - mkey)

        in_maps.append({
            "xT": np.ascontiguousarray(x[b].T).astype(bf16),
            "wqk0": wqk[0], "wqk1": wqk[1], "wqk2": wqk[2], "wqk3": wqk[3],
            "wvp": np.ascontiguousarray(wvp).astype(bf16),
            "wop": np.ascontiguousarray(wop).astype(bf16),
            "qkb": bq,
            "bvr": bvr,
            "expb": expb,
            "tril": tril,
            "vones": vones,
        })
    return in_maps


def kernel(x, mask, Wqkv, bqkv, Wo, bo, _trace=False):
    from concourse.bass_utils import run_bass_kernel_spmd

    nc = build_program()
    in_maps = make_inputs(x, mask, Wqkv, bqkv, Wo, bo)
    res = run_bass_kernel_spmd(nc, in_maps, core_ids=list(range(NCORES)),
                               trace=_trace, trace_cores=[0] if _trace else None)
    bo = np.asarray(bo, dtype=np.float32)
    out = np.empty((B, S, D), dtype=np.float32)
    for b in range(B):
        out[b] = (res.results[b]["out_p"].astype(np.float32)
                  + res.results[b + 4]["out_p"].astype(np.float32) + bo)
    if _trace:
        kernel._last_result = res
    return out


# revision 27
# speedup vs baseline: 1.0496x; 1.0496x over previous
"""Causal self-attention with ALiBi for TRN2, 8 NeuronCores.

Sharding: core c -> batch b = c % 4, head-shard hs = c // 4.
Head-shard hs owns global heads {2j + hs : j in 0..7} (interleaved so both
shards see the same mix of ALiBi slopes -> balanced banded-attention work).

Key HW facts this kernel is built around (measured on TRN2):
  * bf16 matmuls issue at N/2.4GHz cols with LDWEIGHTS overlapped; the PE
    supports row-group tiling (tile_position), so the two heads of a pair
    run their K=64-contraction score matmuls CONCURRENTLY in disjoint
    64-row halves of the array (~1.9x on scores).
  * the HAM clock gate runs the PE at 1.2 GHz until ~3.4us of sustained
    activity and re-gates after any ~3.4us idle window; the kernel keeps
    one continuous PE instruction stream (warm-up burst -> DMA-paced
    d-outer first projection -> paced attention/filler interleave).
  * ACT exp costs (w + 352)/1.2 ns per op; total exp (~124us) is nearly
    co-critical with the PE stream (~160us), so attention score/exp groups
    are spread across the WHOLE kernel, paced against dense matmul filler
    units (remaining projections, V stripes, out-proj stripes).
  * reciprocal_approx_fast CANNOT read PSUM directly (garbage, no error):
    the denominator row is staged through SBUF first.

Per-core computation (B=1 batch, 8 heads):
  V is produced in [s, col] layout with a ones column appended per head
  (the PV matmul then yields both the unnormalized output AND the softmax
  denominator).  Per pair p (slots 2p, 2p+1): Q^T/K^T in [col, s] layout,
  head pairs packed 64+64 into 128-partition tiles (Q pre-scaled by
  1/sqrt(HD) via host-scaled Wq).  Scores S^T[k,q] = K_h^T.T @ Q_h^T with
  64-row tile_position per half, exp on ACT with per-partition bias
  slope*(k - qmid) (band truncation at CUT skips vanishing k-tiles),
  PV accumulation into PSUM [65, 512] over the k band, then normalize
  (reciprocal of row 64, broadcast via gpsimd) into OT (bf16).  Pairs run
  heavy-to-light (3,2,1,0); out-projection stripes interleave into the
  last attention window per q-chunk.

Host side: shard/transpose/bf16-convert/pack inputs, run SPMD on 8 cores,
sum the two head-shards' partial outputs per batch, add bo.
"""

import math

import numpy as np

B, S, D, H = 4, 2048, 1024, 16
HD = D // H
NSLOT = 8          # local heads per core
NQC = 4            # q chunks of 512
NKT = 16           # k tiles of 128
SC = 512
KT = 128
NCORES = 8

# ALiBi slopes for global heads
SLOPES = [2.0 ** (-0.5 * (h + 1)) for h in range(H)]

# band cutoff: terms with slope*(q-k) > CUT are < e^-CUT relative to the
# diagonal term and invisible next to the bf16 matmul noise (~4e-3)
CUT = 8.0


def _bt(h):
    """Band width in 128-k-tiles for global head h (delta_max + 1)."""
    d_max = int(math.ceil(CUT / SLOPES[h]))
    return min(NKT, (127 + d_max) // 128 + 1)


def _w(h):
    """Max exp-op width (q columns) for global head h: slope*(W/2) <= 64
    (bounds the exp dynamic range across a recentered column block)."""
    s = SLOPES[h]
    if s * 256.0 <= 64.0:
        return 512
    if s * 128.0 <= 64.0:
        return 256
    return 128


# per-slot params = union over the two head shards (program is SPMD-shared)
SLOT_BT = [max(_bt(2 * j), _bt(2 * j + 1)) for j in range(NSLOT)]
SLOT_W = [min(_w(2 * j), _w(2 * j + 1)) for j in range(NSLOT)]


def plan_attention():
    """Enumerate all attention tile ops. Returns (ops, bias_cols) where ops is
    a list of dicts and bias_cols maps (slot, mkey) -> expb column index."""
    bias_cols = {}
    ops = []
    for p in range(4):
        for qc in range(NQC):
            for kt in range(4 * qc + 4):
                for half in (0, 1):
                    j = 2 * p + half
                    bt, w = SLOT_BT[j], SLOT_W[j]
                    lo = max(0, 4 * qc - bt + 1)
                    if kt < lo:
                        continue
                    qs_start = max(4 * qc, kt)
                    qs_end = min(4 * qc + 3, kt + bt - 1)
                    if qs_start > qs_end:
                        continue
                    c0 = 128 * (qs_start - 4 * qc)
                    c1 = 128 * (qs_end - 4 * qc) + 128
                    # exp ops aligned to an absolute w-grid within the qc
                    # chunk: qmid (the recentering constant) must depend only
                    # on the column block, never on kt, so that every term
                    # entering a given column's softmax sum carries the same
                    # exp(-slope*qmid) factor.
                    exps = []
                    for g in range((c0 // w) * w, c1, w):
                        a, e = max(c0, g), min(c1, g + w)
                        if a >= e:
                            continue
                        mkey = (512 * qc + g + w // 2) - 128 * kt
                        col = bias_cols.setdefault((j, mkey), len(bias_cols))
                        exps.append((a, e - a, col))
                    ops.append(dict(qc=qc, p=p, half=half, j=j, kt=kt,
                                    c0=c0, c1=c1, exps=exps,
                                    tril=(kt >= 4 * qc),
                                    first=(kt == lo), last=(kt == 4 * qc + 3)))
    return ops, bias_cols


ATT_OPS, BIAS_COLS = plan_attention()
NBIAS = len(BIAS_COLS)

# xT tile consumption order for the DMA-paced startup pass (approximate
# arrival order given the 3-queue round-robin below)
D_ORDER = [1, 2, 4, 5, 0, 6, 7, 3]

_nc_cache = None


def build_program():
    global _nc_cache
    if _nc_cache is not None:
        return _nc_cache

    import concourse.bacc as bacc
    import concourse.tile as tile
    from concourse import mybir

    F32 = mybir.dt.float32
    BF16 = mybir.dt.bfloat16
    EXP = mybir.ActivationFunctionType.Exp
    COPY = mybir.ActivationFunctionType.Copy

    nc = bacc.Bacc("TRN2", target_bir_lowering=False, debug=False,
                   num_devices=NCORES)

    xT_d = nc.dram_tensor("xT", [D, S], BF16, kind="ExternalInput")
    # packed weights: one DMA each.  wqk[p]: [128, d(8) x (Q128 | K128)];
    # wvp: [128, d(8) x 512 vcols]; wop: [128, f(4) x 1024 dcols]
    wqk_d = [nc.dram_tensor(f"wqk{p}", [128, 2048], BF16,
                            kind="ExternalInput") for p in range(4)]
    wvp_d = nc.dram_tensor("wvp", [128, 4096], BF16, kind="ExternalInput")
    wop_d = nc.dram_tensor("wop", [128, 4096], BF16, kind="ExternalInput")
    qkb_d = nc.dram_tensor("qkb", [128, 8], F32, kind="ExternalInput")
    bvr_d = nc.dram_tensor("bvr", [128, 512], F32, kind="ExternalInput")
    expb_d = nc.dram_tensor("expb", [128, max(NBIAS, 1)], F32,
                            kind="ExternalInput")
    tril_d = nc.dram_tensor("tril", [128, 128], BF16, kind="ExternalInput")
    vones_d = nc.dram_tensor("vones", [128, 8], BF16, kind="ExternalInput")
    out_d = nc.dram_tensor("out_p", [S, D], BF16, kind="ExternalOutput")

    ops_by_pqc = {}
    for o in ATT_OPS:
        ops_by_pqc.setdefault((o["p"], o["qc"]), []).append(o)

    with tile.TileContext(nc) as tc:
        with nc.allow_low_precision(reason="bf16 attention kernel"), \
             tc.tile_pool(name="persist", bufs=1) as pp, \
             tc.tile_pool(name="expsp", bufs=8) as expsp, \
             tc.tile_pool(name="rcp", bufs=2) as rcp, \
             tc.tile_pool(name="rbp", bufs=2) as rbp, \
             tc.tile_pool(name="outp", bufs=4) as outp, \
             tc.tile_pool(name="qps", bufs=2, space="PSUM") as qps, \
             tc.tile_pool(name="sps", bufs=4, space="PSUM") as sps, \
             tc.tile_pool(name="ops_", bufs=2, space="PSUM") as ops_:

            # ---- persistent tiles ----
            qkT_Q = [pp.tile([128, S], BF16, name=f"qkTQ{p}") for p in range(4)]
            # packed K stationaries: head h of pair p lives in partitions
            # 64h..64h+63 (the natural projection layout); score matmuls are
            # 64-row tile_position'd so both heads run concurrently.
            kqT = [pp.tile([128, S], BF16, name=f"kqT{p}") for p in range(4)]
            Vbuf = [pp.tile([128, NSLOT * 65], BF16, name=f"vb{t}")
                    for t in range(NKT)]
            OT = [pp.tile([128, S], BF16, name=f"OT{p}") for p in range(4)]
            xT = [pp.tile([128, S], BF16, name=f"xT{d}") for d in range(8)]
            wqk_t = [pp.tile([128, 2048], BF16, name=f"wqk_t{p}")
                     for p in range(4)]
            wv = pp.tile([128, 4096], BF16, name="wv")
            wo_t = pp.tile([128, 4096], BF16, name="wo_t")
            qkb_t = pp.tile([128, 8], F32, name="qkb_t")
            bvr_t = pp.tile([128, 512], F32, name="bvr_t")
            expb_t = pp.tile([128, max(NBIAS, 1)], F32, name="expb_t")
            tril_t = pp.tile([128, 128], BF16, name="tril_t")
            vones_t = pp.tile([128, 8], BF16, name="vones_t")
            warm_t = pp.tile([128, 128], BF16, name="warm_t")
            dume = pp.tile([128, 1], BF16, name="dume")

            # ---- input DMA: xT tiles lead all three queues (they gate the
            # startup projection pass); small tensors follow; wop last.
            nc.gpsimd.dma_start(out=wqk_t[3], in_=wqk_d[3][:, :])
            for d in [0, 3]:
                nc.gpsimd.dma_start(out=xT[d], in_=xT_d[128 * d:128 * (d + 1), :])
            for d in [1, 4, 6]:
                nc.scalar.dma_start(out=xT[d], in_=xT_d[128 * d:128 * (d + 1), :])
            for d in [2, 5, 7]:
                nc.sync.dma_start(out=xT[d], in_=xT_d[128 * d:128 * (d + 1), :])
            nc.gpsimd.dma_start(out=qkb_t, in_=qkb_d[:, :])
            nc.gpsimd.dma_start(out=expb_t, in_=expb_d[:, :])
            nc.gpsimd.dma_start(out=vones_t, in_=vones_d[:, :])
            nc.gpsimd.dma_start(out=tril_t, in_=tril_d[:, :])
            nc.sync.dma_start(out=wv, in_=wvp_d[:, :])
            nc.gpsimd.dma_start(out=bvr_t, in_=bvr_d[:, :])
            nc.scalar.dma_start(out=wqk_t[2], in_=wqk_d[2][:, :])
            nc.sync.dma_start(out=wqk_t[1], in_=wqk_d[1][:, :])
            nc.gpsimd.dma_start(out=wqk_t[0], in_=wqk_d[0][:, :])
            nc.scalar.dma_start(out=wo_t, in_=wop_d[:, :])

            # ---- PE warm-up on a memset tile (no DMA dependency): keeps the
            # HAM clock gate fed from ~0.4us until the first xT tiles land.
            nc.vector.memset(warm_t, 0.0)
            for _ in range(24):
                wps = qps.tile([128, 128], F32, name="warm", tag="q")
                nc.tensor.matmul(wps, warm_t, warm_t, start=True, stop=True)
            # preload the ACT exp table set during the DMA window
            nc.scalar.activation(dume, warm_t[:, 0:1], EXP,
                                 bias=0.0, scale=1.0)

            # ones columns of Vbuf (col 64 of each 65-wide head group)
            for t in range(NKT):
                ones_view = Vbuf[t].rearrange("p (h c) -> p h c", c=65)[:, :, 64:65]
                nc.vector.tensor_copy(ones_view, vones_t.unsqueeze(2))

            # ================= emission units =================

            def proj_fin(p, m, si_abs, psq):
                """Move one [128,512] projection PSUM tile to SBUF (+bias)."""
                if m == 0:
                    nc.vector.tensor_scalar_add(
                        qkT_Q[p][:, SC * si_abs:SC * (si_abs + 1)], psq,
                        qkb_t[:, p:p + 1])
                else:
                    nc.vector.tensor_scalar_add(
                        kqT[p][:, SC * si_abs:SC * (si_abs + 1)], psq,
                        qkb_t[:, 4 + p:5 + p])

            def proj_pass_start(p, si_abs):
                """DMA-paced d-outer Q+K projection for s-chunk si_abs: both
                tiles accumulate as each xT[d] lands (startup only)."""
                psqK = qps.tile([128, SC], F32, name="psqK", tag="q")
                psqQ = qps.tile([128, SC], F32, name="psqQ", tag="q")
                for i, d in enumerate(D_ORDER):
                    mv = xT[d][:, SC * si_abs:SC * (si_abs + 1)]
                    nc.tensor.matmul(
                        psqK, wqk_t[p][:, 256 * d + 128:256 * d + 256], mv,
                        start=(i == 0), stop=(i == 7), skip_group_check=True)
                    nc.tensor.matmul(
                        psqQ, wqk_t[p][:, 256 * d:256 * d + 128], mv,
                        start=(i == 0), stop=(i == 7), skip_group_check=True)
                    # keep the HAM activity monitor fed through the DMA-paced
                    # gaps between xT arrivals (sustained-busy -> 2.4 GHz by
                    # the time attention starts)
                    if i >= 1:
                        # ops_ pool is idle during startup; its banks host the
                        # gap-filling warm tiles (psq tiles hold qps' slots).
                        # These only run when the next xT tile hasn't landed
                        # (the scheduler orders by readiness), so oversizing
                        # is cheap.
                        for _ in range(4):
                            wps = ops_.tile([128, 128], F32, name="warm2",
                                            tag="psumO")
                            nc.tensor.matmul(wps, warm_t, warm_t,
                                             start=True, stop=True)
                proj_fin(p, 1, si_abs, psqK)
                proj_fin(p, 0, si_abs, psqQ)

            def proj_unit(p, m, si_abs):
                """Classic d-inner projection unit: one [128,512] output."""
                psq = qps.tile([128, SC], F32, name="psq", tag="q")
                for d in range(8):
                    nc.tensor.matmul(
                        psq, wqk_t[p][:, 256 * d + 128 * m:256 * d + 128 * (m + 1)],
                        xT[d][:, SC * si_abs:SC * (si_abs + 1)],
                        start=(d == 0), stop=(d == 7))
                proj_fin(p, m, si_abs, psq)

            def v_unit(st):
                """V projection stripe st into Vbuf[st] (bf16, [s,col])."""
                psv = qps.tile([128, SC], F32, name="psv", tag="q")
                for d in range(8):
                    nc.tensor.matmul(
                        psv, xT[d][:, 128 * st:128 * (st + 1)],
                        wv[:, 512 * d:512 * (d + 1)],
                        start=(d == 0), stop=(d == 7))
                vdst = Vbuf[st].rearrange("p (h c) -> p h c", c=65)[:, :, 0:64]
                nc.vector.tensor_tensor(
                    vdst, psv.rearrange("p (g c) -> p g c", c=64),
                    bvr_t.rearrange("p (g c) -> p g c", c=64),
                    op=mybir.AluOpType.add)

            def out_unit(st, act_free):
                """Out-projection stripe st: psum accumulate over 4 pairs,
                then copy+DMA per 512-col half."""
                pse = [qps.tile([128, SC], F32, name="pse", tag="q")
                       for _ in range(2)]
                for d in range(4):
                    for e in range(2):
                        nc.tensor.matmul(
                            pse[e], OT[d][:, 128 * st:128 * (st + 1)],
                            wo_t[:, 1024 * d + SC * e:1024 * d + SC * (e + 1)],
                            start=(d == 0), stop=(d == 3))
                for e in range(2):
                    ob = outp.tile([128, SC], BF16, name="ob")
                    if act_free and e == 1:
                        nc.scalar.activation(ob, pse[e], COPY)
                    else:
                        nc.vector.tensor_copy(ob, pse[e])
                    eng = nc.gpsimd if (st + e) % 2 == 0 else nc.sync
                    eng.dma_start(
                        out=out_d[128 * st:128 * (st + 1),
                                  SC * e:SC * (e + 1)],
                        in_=ob)

            tril_ctr = [0]

            def emit_scores(grp):
                """Row-tiled score matmuls + exp for one kt group (both
                halves concurrent in disjoint 64-row PE tiles)."""
                out = []
                p = grp[0]["p"]
                for o in grp:
                    half, kt = o["half"], o["kt"]
                    c0, c1 = o["c0"], o["c1"]
                    qc = o["qc"]
                    hb = 64 * half
                    psS = sps.tile([128, SC], F32, name="psS")
                    nc.tensor.matmul(
                        psS[:, c0:c1],
                        kqT[p][hb:hb + 64, 128 * kt:128 * (kt + 1)],
                        qkT_Q[p][hb:hb + 64, SC * qc + c0:SC * qc + c1],
                        start=True, stop=True)
                    eS = expsp.tile([128, SC], BF16, name="eS")
                    for (a, ww, col) in o["exps"]:
                        nc.scalar.activation(
                            eS[:, a:a + ww], psS[:, a:a + ww], EXP,
                            bias=expb_t[:, col:col + 1], scale=1.0)
                    if o["tril"]:
                        # NOTE: must stay on vector — gpsimd tensor_tensor
                        # lives in a different ucode library than
                        # partition_broadcast, and alternating them costs
                        # ~6.5us per LOAD_LIB swap.
                        nc.vector.tensor_mul(
                            eS[:, c0:c0 + 128], eS[:, c0:c0 + 128], tril_t)
                    out.append((o, eS))
                return out

            def emit_pv(ready, psumO):
                for (o, eS) in ready:
                    c0, c1 = o["c0"], o["c1"]
                    nc.tensor.matmul(
                        psumO[o["half"]][0:65, c0:c1],
                        Vbuf[o["kt"]][:, 65 * o["j"]:65 * o["j"] + 65],
                        eS[:, c0:c1],
                        start=o["first"], stop=o["last"])

            def emit_norm(p, qc, psumO):
                # both halves' chains pipelined: copies, then recips, then
                # broadcasts, then muls (keeps chain latency ~2 stages, not 6)
                ss = {}
                for half in (0, 1):
                    ss[half] = rcp.tile([1, SC], F32, name="ssum")
                    nc.vector.tensor_copy(ss[half], psumO[half][64:65, :])
                rc = {}
                for half in (0, 1):
                    rc[half] = rcp.tile([1, SC], F32, name="rc")
                    nc.vector.reciprocal_approx_fast(rc[half], ss[half])
                rb = {}
                for half in (0, 1):
                    rb[half] = rbp.tile([64, SC], F32, name="rb")
                    nc.gpsimd.partition_broadcast(rb[half], rc[half])
                for half in (0, 1):
                    nc.vector.tensor_mul(
                        OT[p][64 * half:64 * half + 64,
                              SC * qc:SC * (qc + 1)],
                        psumO[half][0:64, :],
                        rb[half])

            # ================= cost model for pacing =================

            def grp_costs(grp):
                pe = 0.0
                act = 0.0
                bycols = {0: 0, 1: 0}
                for o in grp:
                    bycols[o["half"]] = o["c1"] - o["c0"]
                    for (a, ww, col) in o["exps"]:
                        act += (ww + 352) / 1.2
                pe = max(bycols.values()) / 2.4 + 110.0
                return pe, act

            def pv_costs(grp):
                return sum(o["c1"] - o["c0"] for o in grp) / 2.4 + 60.0 * len(grp)

            UNIT_PE = 8 * (512 / 2.4 + 45.0)   # proj/v/out unit estimate

            # ================= filler list =================
            # (emit_fn, pe_ns); consumed in order by the pacer
            fillers = []
            for si_abs in (1, 2, 3):
                fillers.append((lambda s=si_abs: proj_unit(3, 1, s), UNIT_PE))
                fillers.append((lambda s=si_abs: proj_unit(3, 0, s), UNIT_PE))
            for st in range(NKT):
                fillers.append((lambda s=st: v_unit(s), UNIT_PE))
            for p_ in (2, 1, 0):
                for si_abs in range(4):
                    fillers.append((lambda pp_=p_, s=si_abs: proj_unit(pp_, 1, s),
                                    UNIT_PE))
                    fillers.append((lambda pp_=p_, s=si_abs: proj_unit(pp_, 0, s),
                                    UNIT_PE))

            # filler force-markers
            FPROJ3 = 6            # proj pair-3 remainder
            FV = FPROJ3 + NKT     # V stripes end index (6..21)
            FP2 = FV + 8
            FP1 = FP2 + 8
            FP0 = FP1 + 8

            state = dict(fi=0, pe=0.0, act=1.0)

            # overall PE/ACT balance ratio for the pacer
            tot_act = sum(grp_costs([o])[1] for o in ATT_OPS)
            tot_pe = sum(f[1] for f in fillers) + 16 * UNIT_PE
            for (pq, opl) in ops_by_pqc.items():
                gs = {}
                for o in opl:
                    gs.setdefault(o["kt"], []).append(o)
                for g in gs.values():
                    tot_pe += grp_costs(g)[0] + pv_costs(g)
            RATIO = tot_pe / tot_act

            def force_fill(upto):
                while state["fi"] < upto:
                    fn, pe_ns = fillers[state["fi"]]
                    state["fi"] += 1
                    fn()
                    state["pe"] += pe_ns

            def pace_fill():
                while (state["fi"] < len(fillers)
                       and state["pe"] < state["act"] * RATIO):
                    fn, pe_ns = fillers[state["fi"]]
                    state["fi"] += 1
                    fn()
                    state["pe"] += pe_ns

            def attention_qc(p, qc, req_scores, req_pv_base):
                """Emit one (pair, qc) attention chunk through the pacer."""
                opl = ops_by_pqc.get((p, qc), [])
                groups = []
                for o in opl:
                    if groups and groups[-1][0]["kt"] == o["kt"]:
                        groups[-1].append(o)
                    else:
                        groups.append([o])
                psumO = {h: ops_.tile([65, SC], F32, name="psumO")
                         for h in (0, 1)}
                PIPE = 3
                pend = []
                force_fill(req_scores)
                for grp in groups:
                    pace_fill()
                    pend.append(emit_scores(grp))
                    pe, act = grp_costs(grp)
                    state["pe"] += pe
                    state["act"] += act
                    if len(pend) > PIPE:
                        ready = pend.pop(0)
                        if req_pv_base is not None:
                            force_fill(min(req_pv_base + ready[0][0]["kt"] + 1,
                                           FV))
                        pace_fill()
                        emit_pv(ready, psumO)
                        state["pe"] += pv_costs([o for (o, _) in ready])
                for ready in pend:
                    if req_pv_base is not None:
                        force_fill(min(req_pv_base + ready[0][0]["kt"] + 1, FV))
                    pace_fill()
                    emit_pv(ready, psumO)
                    state["pe"] += pv_costs([o for (o, _) in ready])
                emit_norm(p, qc, psumO)

            # ================= schedule =================

            # DMA-paced startup: Q+K projection of pair 3 for s-chunk 0
            proj_pass_start(3, 0)

            # pair 3 window (fillers: proj3 remainder, V stripes, proj2...)
            REQ3 = {0: 0, 1: 2, 2: 4, 3: 6}
            for qc in range(NQC):
                attention_qc(3, qc, REQ3[qc], FPROJ3)
            # pair 2 window
            for qc in range(NQC):
                attention_qc(2, qc, FP2, None)
            # pairs 1+0 interleaved by qc; qc0 (the lightest chunk) runs LAST
            # so the tail chain (exp->PV->norm->out) is as short as possible,
            # with the previous qc's out stripes as PE fill between the two
            # pairs of the next qc.
            QC_ORDER = [1, 2, 3, 0]
            prev_qc = None
            for qc in QC_ORDER:
                attention_qc(1, qc, FP1, None)
                if prev_qc is not None:
                    for st in range(4 * prev_qc, 4 * prev_qc + 2):
                        out_unit(st, act_free=False)
                        state["pe"] += UNIT_PE
                attention_qc(0, qc, FP0, None)
                if prev_qc is not None:
                    for st in range(4 * prev_qc + 2, 4 * prev_qc + 4):
                        out_unit(st, act_free=False)
                        state["pe"] += UNIT_PE
                prev_qc = qc
            for st in range(0, 4):
                out_unit(st, act_free=True)
                state["pe"] += UNIT_PE
            force_fill(len(fillers))

    nc.compile()
    _nc_cache = nc
    return nc


def make_inputs(x, mask, Wqkv, bqkv, Wo, bo):
    """Build the 8 per-core input maps."""
    import ml_dtypes

    bf16 = ml_dtypes.bfloat16
    x = np.asarray(x, dtype=np.float32)
    Wqkv = np.asarray(Wqkv, dtype=np.float32)
    bqkv = np.asarray(bqkv, dtype=np.float32)
    Wo = np.asarray(Wo, dtype=np.float32)

    # diagonal-block mask in [k_partition, q_column] layout: keep k <= q,
    # i.e. partition p <= column c -> UPPER-triangular
    tril = np.triu(np.ones((128, 128), dtype=bf16))
    vones = np.ones((128, 8), dtype=bf16)
    p_idx = np.arange(128, dtype=np.float32)[:, None]

    in_maps = []
    for c in range(NCORES):
        b, hs = c % 4, c // 4
        heads = [2 * j + hs for j in range(NSLOT)]

        # per-pair packed QK weights: [128, d(8) x (Q128 | K128)]
        # pair p covers slots 2p (partitions 0-63) and 2p+1 (64-127).
        wqk = {}
        for p in range(4):
            h0, h1 = heads[2 * p], heads[2 * p + 1]
            qcols = np.concatenate(
                [np.arange(h0 * HD, h0 * HD + HD),
                 np.arange(h1 * HD, h1 * HD + HD)])
            kcols = D + qcols
            wq = Wqkv[:, qcols] * 0.125            # [D, 128]
            wk = Wqkv[:, kcols]
            pack = np.empty((128, 2048), dtype=np.float32)
            for d in range(8):
                pack[:, 256 * d:256 * d + 128] = wq[128 * d:128 * (d + 1)]
                pack[:, 256 * d + 128:256 * d + 256] = wk[128 * d:128 * (d + 1)]
            wqk[p] = np.ascontiguousarray(pack).astype(bf16)

        # packed V weights [128, d(8) x 512] and Wo [128, f(4) x 1024]
        vcols = np.concatenate(
            [np.arange(2 * D + h * HD, 2 * D + h * HD + HD) for h in heads])
        wvm = Wqkv[:, vcols]                       # [D, 512]
        wvp = np.empty((128, 4096), dtype=np.float32)
        for d in range(8):
            wvp[:, 512 * d:512 * (d + 1)] = wvm[128 * d:128 * (d + 1)]
        rows = np.concatenate(
            [np.arange(h * HD, h * HD + HD) for h in heads])
        wom = Wo[rows, :]                          # [512, 1024]
        wop = np.empty((128, 4096), dtype=np.float32)
        for d in range(4):
            wop[:, 1024 * d:1024 * (d + 1)] = wom[128 * d:128 * (d + 1)]

        # biases: qkb col m -> per-partition bias for (Q pairs 0-3, K pairs
        # 0-3); pair p partitions = slot 2p dims then slot 2p+1 dims
        bq = np.empty((128, 8), dtype=np.float32)
        for p in range(4):
            h0, h1 = heads[2 * p], heads[2 * p + 1]
            qb = np.concatenate([bqkv[h0 * HD:h0 * HD + HD],
                                 bqkv[h1 * HD:h1 * HD + HD]]) * 0.125
            kb = np.concatenate([bqkv[D + h0 * HD:D + h0 * HD + HD],
                                 bqkv[D + h1 * HD:D + h1 * HD + HD]])
            bq[:, p] = qb
            bq[:, 4 + p] = kb
        bvr = np.broadcast_to(bqkv[2 * D:][vcols - 2 * D], (128, 512)).copy()

        expb = np.zeros((128, max(NBIAS, 1)), dtype=np.float32)
        for (j, mkey), col in BIAS_COLS.items():
            expb[:, col:col + 1] = SLOPES[2 * j + hs] * (p_idx - mkey)

        in_maps.append({
            "xT": np.ascontiguousarray(x[b].T).astype(bf16),
            "wqk0": wqk[0], "wqk1": wqk[1], "wqk2": wqk[2], "wqk3": wqk[3],
            "wvp": np.ascontiguousarray(wvp).astype(bf16),
            "wop": np.ascontiguousarray(wop).astype(bf16),
            "qkb": bq,
            "bvr": bvr,
            "expb": expb,
            "tril": tril,
            "vones": vones,
        })
    return in_maps


def kernel(x, mask, Wqkv, bqkv, Wo, bo, _trace=False):
    from concourse.bass_utils import run_bass_kernel_spmd

    nc = build_program()
    in_maps = make_inputs(x, mask, Wqkv, bqkv, Wo, bo)
    res = run_bass_kernel_spmd(nc, in_maps, core_ids=list(range(NCORES)),
                               trace=_trace, trace_cores=[0] if _trace else None)
    bo = np.asarray(bo, dtype=np.float32)
    out = np.empty((B, S, D), dtype=np.float32)
    for b in range(B):
        out[b] = (res.results[b]["out_p"].astype(np.float32)
                  + res.results[b + 4]["out_p"].astype(np.float32) + bo)
    if _trace:
        kernel._last_result = res
    return out
